# revision 14
# baseline (speedup 1.0000x reference)
"""Euler-Maruyama SDE paths on Trainium2 (Bass/Tile, 8 NeuronCores).

Recurrence: Z[:, t] = Z[:, t-1] * (1 + r*dt + s*sqrt(dt)*W[:, t]), Z[:, 0] = Z0.

Sharding: batch (path) dim split evenly across the 8 cores (pure data
parallel); the time recurrence stays on-core; weights are baked as immediates.

Per-core pipeline, tiled [128 partitions x R rows x (NT+1) cols]:
  1. in-DMA full contiguous W rows (sync HWDGE queue) — loading the unused
     col 0 keeps each partition's packet one contiguous span (26 GB/s/engine
     vs 18 for the strided W[:,1:] load).
  2. ACT affine in place: M = scale*W + bias.
  3. GPSIMD pair products into the output tile's odd cols:
     E[k] = M[2k+1]*M[2k+2].
  4. DVE tensor_tensor_scan over E (one per row, initial=Z0) -> even output
     cols. The scan runs at ~2.3 ns/elem (serial recurrence), so halving its
     length via pairs moved the critical path back to DMA.
  5. DVE tensor_tensor recovers odd outputs Z[2k+1] = Z[2k]*M[2k+1],
     overwriting the consumed E values.
  6. out-DMA on the gpsimd software-DGE queue (never blocks in-DMA prefetch).

Measured ~373-390 us on hardware (baseline single-scan version: ~422-444 us);
the wall is one straggler DMA engine (eng 79, ~22.5-23.7 GB/s vs 26.8 for the
other 15, uniform round-robin packet distribution, not controllable).
"""

import numpy as np

import concourse.bacc as bacc
import concourse.mybir as mybir
import concourse.tile as tile
from concourse.bass_utils import run_bass_kernel_spmd

N_CORES = 8
B = 131072
NT = 1024  # time steps; output has NT+1 columns
ROWS = B // N_CORES  # 16384 rows per core
P = 128  # SBUF partitions
R = 4  # rows per partition per tile
G = ROWS // (P * R)  # tiles per core

F32 = mybir.dt.float32


def _build_nc(rows: int, nt: int, r: float, s: float, rpp: int,
              w_bufs: int = 6, o_bufs: int = 6):
    """Build the per-core Bass program. rows = batch rows on this core,
    nt = time steps, rpp = rows per partition per tile."""
    dt = np.float32(1.0 / nt)
    sdt = np.float32(np.sqrt(dt))
    scale = float(np.float32(s) * sdt)  # multiplies W
    bias = float(np.float32(1.0) + np.float32(r) * dt)

    g = rows // (P * rpp)
    assert rows == P * rpp * g

    nc = bacc.Bacc("TRN2", target_bir_lowering=False, debug=False,
                   num_devices=N_CORES)
    W = nc.dram_tensor("W", [rows, nt + 1], F32, kind="ExternalInput").ap()
    Z0 = nc.dram_tensor("Z0", [rows], F32, kind="ExternalInput").ap()
    Z = nc.dram_tensor("Z", [rows, nt + 1], F32, kind="ExternalOutput").ap()

    # row = p*(rpp*g) + m, m = position within partition; tile t covers
    # m in [t*rpp, (t+1)*rpp). Uniform rpp-sized chunks measured fastest
    # (variable-size chunks and rpp=8 both regressed: the straggler DMA
    # engine's deficit grows with packet-count changes in either
    # direction, and small tiles inflate DVE scheduling bubbles).
    chunks = [rpp] * g

    W_v = W.rearrange("(p m) c -> p m c", p=P)
    Z_v = Z.rearrange("(p m) c -> p m c", p=P)
    Z0_v = Z0.rearrange("(p m) -> p m", p=P)  # [P, rpp*g]

    with tile.TileContext(nc) as tc:
        with (
            tc.tile_pool(name="z0", bufs=1) as z0_pool,
            tc.tile_pool(name="w", bufs=w_bufs) as w_pool,
            tc.tile_pool(name="o", bufs=o_bufs) as o_pool,
        ):
            z0_all = z0_pool.tile([P, rpp * g], F32)
            # z0 preload on the scalar HWDGE queue so the first W-tile
            # load (sync queue) isn't queued behind its 128 packets
            nc.scalar.dma_start(z0_all[:], Z0_v[:])
            bias_t = z0_pool.tile([P, 1], F32, tag="bias")
            nc.vector.memset(bias_t[:], bias)

            r0 = 0
            for rt in chunks:
                wt = w_pool.tile([P, rpp, nt + 1], F32, tag="w")
                ot = o_pool.tile([P, rpp, nt + 1], F32, tag="o")
                # load FULL W rows (incl. unused col 0) so each partition's
                # packet is one contiguous rt*(nt+1)*4-byte span — the
                # strided W[:,1:] load ran the in-DMA engines at 18 GB/s
                # vs 22.4+ for contiguous.
                nc.sync.dma_start(wt[:, :rt, :], W_v[:, r0:r0 + rt, :])
                # M = scale*W + bias in place on cols 1.. (ACT engine)
                nc.scalar.activation(
                    wt[:, :rt, 1:], wt[:, :rt, 1:],
                    mybir.ActivationFunctionType.Identity,
                    bias=bias_t[:], scale=scale,
                )
                # Z[:, 0] = Z0 (ACT copy, tiny)
                nc.scalar.copy(ot[:, :rt, 0:1],
                               z0_all[:, r0:r0 + rt].rearrange(
                                   "p (j c) -> p j c", c=1))
                # Pair products on GPSIMD into ot's odd cols:
                #   E[k] = M[2k+1]*M[2k+2]  ->  ot[:, :, 2k+1]
                # This halves the serial DVE scan length (the scan runs at
                # ~2.3 ns/elem and was the critical path).
                nc.gpsimd.tensor_tensor(
                    out=ot[:, :rt, 1:nt:2],
                    in0=wt[:, :rt, 1:nt:2],
                    in1=wt[:, :rt, 2:nt + 1:2],
                    op=mybir.AluOpType.mult,
                )
                # DVE scan over pairs: ot[:, 2k+2] = Z0 * cumprod(E[0..k])
                for j in range(rt):
                    nc.vector.tensor_tensor_scan(
                        out=ot[:, j, 2:nt + 1:2],
                        data0=ot[:, j, 1:nt:2],
                        data1=ot[:, j, 1:nt:2],
                        initial=z0_all[:, r0 + j: r0 + j + 1],
                        op0=mybir.AluOpType.mult,
                        op1=mybir.AluOpType.bypass,
                    )
                # Odd outputs: Z[2k+1] = Z[2k] * M[2k+1] (DVE tensor_tensor,
                # overwrites the consumed E values in ot's odd cols)
                nc.vector.tensor_tensor(
                    out=ot[:, :rt, 1:nt:2],
                    in0=ot[:, :rt, 0:nt - 1:2],
                    in1=wt[:, :rt, 1:nt:2],
                    op=mybir.AluOpType.mult,
                )
                # out-DMAs issue on the gpsimd sequencer so they never
                # block in-DMA prefetch on sync
                nc.gpsimd.dma_start(Z_v[:, r0:r0 + rt, :], ot[:, :rt, :])
                r0 += rt

    nc.compile()
    return nc


_NC_CACHE: dict = {}


def _get_nc(r: float, s: float):
    key = (r, s)
    if key not in _NC_CACHE:
        _NC_CACHE[key] = _build_nc(ROWS, NT, r, s, R)
    return _NC_CACHE[key]


_JIT_CACHE: dict = {}


def _get_sharded_fn(nc):
    """Build a jit(shard_map) callable for the per-core Bass program, with
    inputs expected already device-placed.  Mirrors
    concourse.bass2jax.run_bass_via_pjrt, but lets us pre-place inputs so
    no host->device traffic overlaps (and steals HBM bandwidth from) the
    kernel execution."""
    if id(nc) in _JIT_CACHE:
        return _JIT_CACHE[id(nc)]

    import jax
    from jax.sharding import Mesh, NamedSharding, PartitionSpec
    from jax.experimental.shard_map import shard_map

    from concourse import bass2jax
    from concourse.bass2jax import _bass_exec_p, partition_id_tensor

    bass2jax.install_neuronx_cc_hook()

    partition_name = (nc.partition_id_tensor.name
                      if nc.partition_id_tensor else None)
    in_names, out_names, out_avals = [], [], []
    for alloc in nc.m.functions[0].allocations:
        if not isinstance(alloc, mybir.MemoryLocationSet):
            continue
        name = alloc.memorylocations[0].name
        if alloc.kind == "ExternalInput":
            if name != partition_name:
                in_names.append(name)
        elif alloc.kind == "ExternalOutput":
            out_names.append(name)
            out_avals.append(jax.core.ShapedArray(
                tuple(alloc.tensor_shape), mybir.dt.np(alloc.dtype)))
    n_params = len(in_names)
    all_in_names = list(in_names) + list(out_names)
    if partition_name is not None:
        all_in_names.append(partition_name)

    def _body(*args):
        operands = list(args)
        if partition_name is not None:
            operands.append(partition_id_tensor())
        outs = _bass_exec_p.bind(
            *operands,
            out_avals=tuple(out_avals),
            in_names=tuple(all_in_names),
            out_names=tuple(out_names),
            lowering_input_output_aliases=(),
            sim_require_finite=True,
            sim_require_nnan=True,
            nc=nc,
        )
        return tuple(outs)

    devices = jax.devices()[:N_CORES]
    mesh = Mesh(np.asarray(devices), ("core",))
    sharding = NamedSharding(mesh, PartitionSpec("core"))
    n_outs = len(out_avals)
    donate = tuple(range(n_params, n_params + n_outs))
    sharded = jax.jit(
        shard_map(_body, mesh=mesh,
                  in_specs=(PartitionSpec("core"),) * (n_params + n_outs),
                  out_specs=(PartitionSpec("core"),) * n_outs,
                  check_rep=False),
        donate_argnums=donate, keep_unused=True,
    )
    # device-side zero alloc for donated output buffers (no H2D transfer)
    zeros_fn = jax.jit(
        lambda: tuple(
            jax.numpy.zeros((N_CORES * a.shape[0], *a.shape[1:]), a.dtype)
            for a in out_avals),
        out_shardings=tuple(sharding for _ in out_avals),
    )
    entry = (sharded, zeros_fn, in_names, out_names, out_avals, sharding)
    _JIT_CACHE[id(nc)] = entry
    return entry


def run(Z0, W, Wf, Wg, profile_ctx=None):
    import jax

    Z0 = np.ascontiguousarray(np.asarray(Z0, dtype=np.float32))
    W = np.ascontiguousarray(np.asarray(W, dtype=np.float32))
    r = float(np.asarray(Wf, dtype=np.float32)[0, 0])
    s = float(np.asarray(Wg, dtype=np.float32)[0, 0])
    nc = _get_nc(r, s)
    sharded, zeros_fn, in_names, out_names, out_avals, sharding = \
        _get_sharded_fn(nc)

    host_in = {"W": W, "Z0": Z0}
    # pre-place inputs + donated zero outputs on device, block before launch
    # (so no host->device streaming steals HBM bandwidth mid-kernel)
    dev_in = [jax.device_put(host_in[n], sharding) for n in in_names]
    dev_zeros = list(zeros_fn())
    jax.block_until_ready(dev_in + dev_zeros)

    if profile_ctx is not None:
        with profile_ctx:
            outs = jax.block_until_ready(sharded(*dev_in, *dev_zeros))
    else:
        outs = jax.block_until_ready(sharded(*dev_in, *dev_zeros))

    out_map = dict(zip(out_names, outs))
    Z = np.asarray(out_map["Z"])
    return (Z, W), nc


def _run_fallback(Z0, W, Wf, Wg):
    """Stock dispatch via run_bass_kernel_spmd, in case the pre-placed
    jit/shard_map path hits an incompatibility."""
    Z0 = np.ascontiguousarray(np.asarray(Z0, dtype=np.float32))
    W = np.ascontiguousarray(np.asarray(W, dtype=np.float32))
    r = float(np.asarray(Wf, dtype=np.float32)[0, 0])
    s = float(np.asarray(Wg, dtype=np.float32)[0, 0])
    nc = _get_nc(r, s)
    in_maps = [
        {"W": W[c * ROWS:(c + 1) * ROWS], "Z0": Z0[c * ROWS:(c + 1) * ROWS]}
        for c in range(N_CORES)
    ]
    res = run_bass_kernel_spmd(nc, in_maps, list(range(N_CORES)))
    Z = np.concatenate([res.results[c]["Z"] for c in range(N_CORES)], axis=0)
    return Z, W


def kernel(Z0, W, Wf, Wg):
    try:
        (Z, W_out), _ = run(Z0, W, Wf, Wg)
    except Exception:
        Z, W_out = _run_fallback(Z0, W, Wf, Wg)
    return Z, W_out

